# revision 21
# baseline (speedup 1.0000x reference)
"""Single-head causal attention (B=8, T=2048, C=1024, H=128) on 8 TRN2 NeuronCores.

Sharding: data-parallel over batch — core b computes batch element b entirely
(no collectives). Host pre-transposes x[b] to xT=[C,T]; the device returns
out^T=[H,T] which the host transposes back.

v2 vs baseline:
  - bk dropped (softmax shift-invariant); bv kept in the v copy (sum p = 1).
  - denominator: GpSimd accumulates exp tiles in f32; one ones-matmul/chunk.
  - causal trim: partial-width score/exp/AV on diagonal tiles; mask is a
    single 128x128 tril multiply on the partial block only.
  - v transposed via DMA XBAR (bf16) instead of PE matmul-transposes.
  - chunk j+1 projection matmuls interleaved into chunk j attention steps to
    cover the exp-latency stalls in the in-order PE queue.
  - k PSUM->SBUF copies on GpSimd; output normalize split for DMA overlap.
"""

import os
import numpy as np

T, C, H = 2048, 1024, 128
B = 8
P = 128
CT = C // P          # 8 contraction tiles
NCH = 4              # t-chunks
CHW = T // NCH       # 512 chunk width
SPC = CHW // P       # 4 s-tiles per chunk
N_CORES = 8
WARMUP = 24

LAST_EXEC_TIME_NS = None

_BUILT = None


def _build():
    global _BUILT
    if _BUILT is not None:
        return _BUILT

    import concourse.bass as bass  # noqa: F401
    import concourse.mybir as mybir
    from concourse import bacc
    from concourse.tile import TileContext

    F32 = mybir.dt.float32
    F32R = mybir.dt.float32r
    BF16 = mybir.dt.bfloat16
    Identity = mybir.ActivationFunctionType.Identity
    Exp = mybir.ActivationFunctionType.Exp
    Mult = mybir.AluOpType.mult
    Add = mybir.AluOpType.add

    nc = bacc.Bacc()

    xT_ext = nc.declare_dram_parameter("xT", [C, T], F32R, isOutput=False)
    w_ext = {
        n: nc.declare_dram_parameter(n, [C, H], F32R, isOutput=False)
        for n in ("Wq", "Wk", "Wv")
    }
    b_ext = {
        n: nc.declare_dram_parameter(n, [H, 1], F32, isOutput=False)
        for n in ("bq", "bv")
    }
    tril_ext = nc.declare_dram_parameter("tril", [P, P], BF16, isOutput=False)
    ones_ext = nc.declare_dram_parameter("ones", [P, P], F32R, isOutput=False)
    ident_ext = nc.declare_dram_parameter("ident", [P, P], BF16, isOutput=False)
    out_ext = nc.declare_dram_parameter("out", [H, T], F32, isOutput=True)

    xT_r = xT_ext.rearrange("(ct p) t -> p ct t", p=P)
    w_r = {n: w_ext[n].rearrange("(ct p) h -> p ct h", p=P) for n in w_ext}

    with TileContext(nc) as tc:
        with (
            tc.tile_pool(name="const", bufs=1) as const,
            tc.tile_pool(name="kt", bufs=NCH) as kt_pool,
            tc.tile_pool(name="vnat", bufs=16) as v_pool,
            tc.tile_pool(name="xch", bufs=2) as x_pool,
            tc.tile_pool(name="qv", bufs=2) as qv_pool,
            tc.tile_pool(name="ex", bufs=6) as e_pool,
            tc.tile_pool(name="dacc", bufs=2) as acc_pool,
            tc.tile_pool(name="outp", bufs=2) as out_pool,
            tc.tile_pool(name="ps_proj", bufs=2, space="PSUM") as proj_ps,
            tc.tile_pool(name="ps_sc", bufs=2, space="PSUM") as sc_ps,
            tc.tile_pool(name="ps_o", bufs=2, space="PSUM") as o_ps,
            tc.tile_pool(name="ps_d", bufs=1, space="PSUM") as d_ps,
            tc.tile_pool(name="ps_tr", bufs=1, space="PSUM") as tr_ps,
        ):
            # ---- constants: weights/bias/mask DMAs, ordered first-needed-first
            w_sb = {}
            for n in ("Wq", "Wk", "Wv"):
                w_sb[n] = [
                    const.tile([P, H], F32R, tag=f"w_{n}_{c}", name=f"w_{n}_{c}")
                    for c in range(CT)
                ]
            b_sb = {
                n: const.tile([H, 1], F32, tag=f"b_{n}", name=f"b_{n}")
                for n in ("bq", "bv")
            }
            x0_tiles = []
            for c in range(CT):
                nc.sync.dma_start(w_sb["Wq"][c][:], w_r["Wq"][:, c, :])
                xt = x_pool.tile([P, CHW], F32R, tag=f"xc{c}", name=f"x0_{c}")
                nc.sync.dma_start(xt[:], xT_r[:, c, 0:CHW])
                x0_tiles.append(xt)
            nc.sync.dma_start(b_sb["bq"][:], b_ext["bq"][:])
            for c in range(CT):
                nc.sync.dma_start(w_sb["Wk"][c][:], w_r["Wk"][:, c, :])
            for c in range(CT):
                nc.sync.dma_start(w_sb["Wv"][c][:], w_r["Wv"][:, c, :])
            nc.sync.dma_start(b_sb["bv"][:], b_ext["bv"][:])
            tril = const.tile([P, P], BF16, tag="tril")
            nc.sync.dma_start(tril[:], tril_ext[:])
            ones_r = const.tile([P, P], F32R, tag="ones_r")
            nc.sync.dma_start(ones_r[:], ones_ext[:])
            ident = const.tile([P, P], BF16, tag="ident")
            nc.sync.dma_start(ident[:], ident_ext[:])
            ones_bf = const.tile([P, P], BF16, tag="ones_bf")
            nc.vector.memset(ones_bf[:], 1.0)

            # PE warmup: dummy matmuls spanning the DMA prologue so HAM is at
            # full clock when the first real matmul issues.
            warm_src = const.tile([P, CHW], BF16, tag="warm_src")
            nc.vector.memset(warm_src[:], 0.0)
            ps_warm = sc_ps.tile([P, CHW], F32, tag="sc", name="ps_warm")
            for _w in range(WARMUP):
                nc.tensor.matmul(
                    ps_warm[:], ones_bf[:], warm_src[:], start=True, stop=True,
                )

            # k chunk tiles [P, CHW] f32r; lhsT slices are [:, 128i:128i+128]
            kt_ch = [None] * NCH
            v_tiles = [None] * (NCH * SPC)
            q_chs = [None] * NCH
            accs = [None] * NCH
            o_banks = [None] * NCH

            def emit_x_dma(j):
                tiles = []
                tsl = slice(CHW * j, CHW * (j + 1))
                for c in range(CT):
                    xt = x_pool.tile([P, CHW], F32R, tag=f"xc{c}", name=f"x{j}_{c}")
                    nc.sync.dma_start(xt[:], xT_r[:, c, tsl])
                    tiles.append(xt)
                return tiles

            def proj_units(j, x_tiles):
                """Thunk list: 24 proj matmuls + copy tails + v transposes."""
                units = []
                ps_tiles = {}

                def mk_mm(kind, wname, c):
                    def f():
                        if c == 0:
                            ps_tiles[kind] = proj_ps.tile(
                                [P, CHW], F32, tag="proj", name=f"ps_{kind}{j}"
                            )
                        nc.tensor.matmul(
                            ps_tiles[kind][:],
                            w_sb[wname][c][:],
                            x_tiles[c][:],
                            start=(c == 0),
                            stop=(c == CT - 1),
                        )
                    return f

                def q_tail():
                    q = qv_pool.tile([P, CHW], F32R, tag="qch", name=f"q{j}")
                    nc.scalar.activation(
                        q[:], ps_tiles["q"][:], Identity, bias=b_sb["bq"][:]
                    )
                    q_chs[j] = q

                def k_tail():
                    kt = kt_pool.tile([P, CHW], F32R, tag="ktch", name=f"kt{j}")
                    nc.vector.tensor_copy(kt[:], ps_tiles["k"][:])
                    kt_ch[j] = kt

                def v_tail():
                    vch = qv_pool.tile([P, CHW], BF16, tag="vch", name=f"v{j}")
                    nc.scalar.activation(
                        vch[:], ps_tiles["v"][:], Identity, bias=b_sb["bv"][:]
                    )
                    ps_tiles["vch"] = vch

                def mk_vt(st):
                    def f():
                        ps_t = tr_ps.tile([P, P], BF16, tag="tr")
                        nc.tensor.transpose(
                            ps_t[:],
                            ps_tiles["vch"][:, P * st : P * (st + 1)],
                            ident[:],
                        )
                        vt = v_pool.tile(
                            [P, P], BF16, tag="vnat", name=f"vnat_{SPC*j+st}"
                        )
                        nc.vector.tensor_copy(vt[:], ps_t[:])
                        v_tiles[SPC * j + st] = vt
                    return f

                for kind, wname, tail in (
                    ("q", "Wq", q_tail),
                    ("k", "Wk", k_tail),
                    ("v", "Wv", v_tail),
                ):
                    for c in range(CT):
                        units.append(mk_mm(kind, wname, c))
                    units.append(tail)
                for st in range(SPC):
                    units.append(mk_vt(st))
                return units

            def emit_attn(j, fill_units):
                """Attention for chunk j, interleaving fill_units into the PE
                stream to cover exp latency."""
                n_s = SPC * (j + 1)
                q_ch = q_chs[j]
                acc = acc_pool.tile([P, CHW], F32R, tag="dacc", name=f"acc{j}")
                accs[j] = acc
                ps_o = o_ps.tile([P, CHW], F32, tag="o", name=f"o{j}")
                o_banks[j] = ps_o

                nu = len(fill_units)
                pending_av = [None]

                def emit_av():
                    if pending_av[0] is not None:
                        pending_av[0]()
                        pending_av[0] = None

                for i in range(n_s):
                    diag = i >= SPC * j
                    st = i - SPC * j
                    w0 = P * st if diag else 0
                    w0sc = min(w0, CHW - 256)

                    ps_sc = sc_ps.tile([P, CHW], F32, tag="sc", name=f"sc{j}_{i}")
                    nc.tensor.matmul(
                        ps_sc[:, w0sc:],
                        kt_ch[i // SPC][:, P * (i % SPC) : P * (i % SPC + 1)],
                        q_ch[:, w0sc:],
                        start=True,
                        stop=True,
                    )
                    eb = e_pool.tile([P, CHW], BF16, tag="e", name=f"e{j}_{i}")
                    nc.scalar.activation(eb[:, w0:], ps_sc[:, w0:], Exp)
                    if diag:
                        nc.vector.tensor_tensor(
                            eb[:, w0 : w0 + P], eb[:, w0 : w0 + P], tril[:], Mult
                        )
                    if i == 0:
                        nc.gpsimd.tensor_copy(acc[:], eb[:])
                    else:
                        nc.gpsimd.tensor_tensor(
                            acc[:, w0:], acc[:, w0:], eb[:, w0:], Add
                        )

                    # fill with next-chunk projection matmuls, then the
                    # previous step's AV (its exp had a full step to finish)
                    lo = nu * i // n_s
                    hi = nu * (i + 1) // n_s
                    for u in range(lo, min(lo + 2, hi)):
                        fill_units[u]()
                    emit_av()
                    for u in range(min(lo + 2, hi), hi):
                        fill_units[u]()

                    vt = v_tiles[i]
                    ii = i

                    def av(eb=eb, vt=vt, w0=w0, ii=ii):
                        nc.tensor.matmul(
                            ps_o[:, w0:],
                            vt[:],
                            eb[:, w0:],
                            start=(ii == 0),
                            stop=(ii == n_s - 1),
                        )
                    pending_av[0] = av
                emit_av()

            def emit_end(j, nsplit):
                tsl0 = CHW * j
                ps_d = d_ps.tile([P, CHW], F32, tag="d", name=f"d{j}")
                nc.tensor.matmul(
                    ps_d[:], ones_r[:], accs[j][:], start=True, stop=True
                )
                recip = out_pool.tile([P, CHW], F32, tag="recip", name=f"rc{j}")
                o_sb = out_pool.tile([P, CHW], F32, tag="osb", name=f"ob{j}")
                w = CHW // nsplit
                for s in range(nsplit):
                    sl = slice(w * s, w * (s + 1))
                    nc.vector.reciprocal_approx_fast(
                        out=recip[:, sl], in_=ps_d[:, sl]
                    )
                    nc.vector.tensor_tensor(
                        o_sb[:, sl], o_banks[j][:, sl], recip[:, sl], Mult
                    )
                    nc.sync.dma_start(
                        out_ext[:, tsl0 + w * s : tsl0 + w * (s + 1)], o_sb[:, sl]
                    )

            # ---- main schedule ----
            x_tiles = x0_tiles
            units = proj_units(0, x_tiles)
            for u in units:
                u()
            for j in range(NCH):
                if j + 1 < NCH:
                    x_next = emit_x_dma(j + 1)
                    fill = proj_units(j + 1, x_next)
                else:
                    fill = []
                emit_attn(j, fill)
                emit_end(j, 2 if j + 1 < NCH else 4)

    nc.compile()
    _BUILT = nc
    return nc


def _host_inputs(x, Wq, bq, Wk, bk, Wv, bv):
    import ml_dtypes

    tril = (np.arange(P)[:, None] <= np.arange(P)[None, :]).astype(
        ml_dtypes.bfloat16
    )
    shared = {
        "Wq": np.ascontiguousarray(Wq, dtype=np.float32),
        "Wk": np.ascontiguousarray(Wk, dtype=np.float32),
        "Wv": np.ascontiguousarray(Wv, dtype=np.float32),
        "bq": np.ascontiguousarray(bq, dtype=np.float32).reshape(H, 1),
        "bv": np.ascontiguousarray(bv, dtype=np.float32).reshape(H, 1),
        "tril": tril,
        "ones": np.ones((P, P), dtype=np.float32),
        "ident": np.eye(P, dtype=np.float32).astype(ml_dtypes.bfloat16),
    }
    in_maps = []
    for b in range(B):
        m = dict(shared)
        m["xT"] = np.ascontiguousarray(np.asarray(x[b], dtype=np.float32).T)
        in_maps.append(m)
    return in_maps


def kernel(x, Wq, bq, Wk, bk, Wv, bv):
    global LAST_EXEC_TIME_NS
    from concourse.bass_utils import run_bass_kernel_spmd

    nc = _build()
    in_maps = _host_inputs(x, Wq, bq, Wk, bk, Wv, bv)
    trace = os.environ.get("BASS_ATTN_TRACE", "0") == "1"
    res = run_bass_kernel_spmd(nc, in_maps, core_ids=list(range(N_CORES)), trace=trace)
    LAST_EXEC_TIME_NS = res.exec_time_ns
    out = np.stack([res.results[b]["out"].T for b in range(B)], axis=0)
    return np.ascontiguousarray(out, dtype=np.float32)


# revision 22
# speedup vs baseline: 1.0062x; 1.0062x over previous
"""Single-head causal attention (B=8, T=2048, C=1024, H=128) on 8 TRN2 NeuronCores.

Sharding: data-parallel over batch — core b computes batch element b entirely
(no collectives). Host pre-transposes x[b] to xT=[C,T]; the device returns
out^T=[H,T] which the host transposes back.

v2 vs baseline:
  - bk dropped (softmax shift-invariant); bv kept in the v copy (sum p = 1).
  - denominator: GpSimd accumulates exp tiles in f32; one ones-matmul/chunk.
  - causal trim: partial-width score/exp/AV on diagonal tiles; mask is a
    single 128x128 tril multiply on the partial block only.
  - v transposed via DMA XBAR (bf16) instead of PE matmul-transposes.
  - chunk j+1 projection matmuls interleaved into chunk j attention steps to
    cover the exp-latency stalls in the in-order PE queue.
  - k PSUM->SBUF copies on GpSimd; output normalize split for DMA overlap.
"""

import os
import numpy as np

T, C, H = 2048, 1024, 128
B = 8
P = 128
CT = C // P          # 8 contraction tiles
NCH = 4              # t-chunks
CHW = T // NCH       # 512 chunk width
SPC = CHW // P       # 4 s-tiles per chunk
N_CORES = 8
WARMUP = 30

LAST_EXEC_TIME_NS = None

_BUILT = None


def _build():
    global _BUILT
    if _BUILT is not None:
        return _BUILT

    import concourse.bass as bass  # noqa: F401
    import concourse.mybir as mybir
    from concourse import bacc
    from concourse.tile import TileContext

    F32 = mybir.dt.float32
    F32R = mybir.dt.float32r
    BF16 = mybir.dt.bfloat16
    Identity = mybir.ActivationFunctionType.Identity
    Exp = mybir.ActivationFunctionType.Exp
    Mult = mybir.AluOpType.mult
    Add = mybir.AluOpType.add

    nc = bacc.Bacc()

    xT_ext = nc.declare_dram_parameter("xT", [C, T], F32R, isOutput=False)
    w_ext = {
        n: nc.declare_dram_parameter(n, [C, H], F32R, isOutput=False)
        for n in ("Wq", "Wk", "Wv")
    }
    b_ext = {
        n: nc.declare_dram_parameter(n, [H, 1], F32, isOutput=False)
        for n in ("bq", "bv")
    }
    tril_ext = nc.declare_dram_parameter("tril", [P, P], BF16, isOutput=False)
    ones_ext = nc.declare_dram_parameter("ones", [P, P], F32R, isOutput=False)
    ident_ext = nc.declare_dram_parameter("ident", [P, P], BF16, isOutput=False)
    out_ext = nc.declare_dram_parameter("out", [H, T], F32, isOutput=True)

    xT_r = xT_ext.rearrange("(ct p) t -> p ct t", p=P)
    w_r = {n: w_ext[n].rearrange("(ct p) h -> p ct h", p=P) for n in w_ext}

    with TileContext(nc) as tc:
        with (
            tc.tile_pool(name="const", bufs=1) as const,
            tc.tile_pool(name="kt", bufs=NCH) as kt_pool,
            tc.tile_pool(name="vnat", bufs=16) as v_pool,
            tc.tile_pool(name="xch", bufs=2) as x_pool,
            tc.tile_pool(name="qv", bufs=2) as qv_pool,
            tc.tile_pool(name="ex", bufs=6) as e_pool,
            tc.tile_pool(name="dacc", bufs=2) as acc_pool,
            tc.tile_pool(name="outp", bufs=2) as out_pool,
            tc.tile_pool(name="ps_proj", bufs=2, space="PSUM") as proj_ps,
            tc.tile_pool(name="ps_sc", bufs=2, space="PSUM") as sc_ps,
            tc.tile_pool(name="ps_o", bufs=2, space="PSUM") as o_ps,
            tc.tile_pool(name="ps_d", bufs=1, space="PSUM") as d_ps,
            tc.tile_pool(name="ps_tr", bufs=1, space="PSUM") as tr_ps,
        ):
            # ---- constants: weights/bias/mask DMAs, ordered first-needed-first
            w_sb = {}
            for n in ("Wq", "Wk", "Wv"):
                w_sb[n] = [
                    const.tile([P, H], F32R, tag=f"w_{n}_{c}", name=f"w_{n}_{c}")
                    for c in range(CT)
                ]
            b_sb = {
                n: const.tile([H, 1], F32, tag=f"b_{n}", name=f"b_{n}")
                for n in ("bq", "bv")
            }
            x0_tiles = []
            for c in range(CT):
                nc.sync.dma_start(w_sb["Wq"][c][:], w_r["Wq"][:, c, :])
                xt = x_pool.tile([P, CHW], F32R, tag=f"xc{c}", name=f"x0_{c}")
                nc.sync.dma_start(xt[:], xT_r[:, c, 0:CHW])
                x0_tiles.append(xt)
            nc.sync.dma_start(b_sb["bq"][:], b_ext["bq"][:])
            for c in range(CT):
                nc.sync.dma_start(w_sb["Wk"][c][:], w_r["Wk"][:, c, :])
            for c in range(CT):
                nc.sync.dma_start(w_sb["Wv"][c][:], w_r["Wv"][:, c, :])
            nc.sync.dma_start(b_sb["bv"][:], b_ext["bv"][:])
            tril = const.tile([P, P], BF16, tag="tril")
            nc.sync.dma_start(tril[:], tril_ext[:])
            ones_r = const.tile([P, P], F32R, tag="ones_r")
            nc.sync.dma_start(ones_r[:], ones_ext[:])
            ident = const.tile([P, P], BF16, tag="ident")
            nc.sync.dma_start(ident[:], ident_ext[:])
            ones_bf = const.tile([P, P], BF16, tag="ones_bf")
            nc.vector.memset(ones_bf[:], 1.0)

            # PE warmup: dummy matmuls spanning the DMA prologue so HAM is at
            # full clock when the first real matmul issues.
            warm_src = const.tile([P, CHW], BF16, tag="warm_src")
            nc.vector.memset(warm_src[:], 0.0)
            ps_warm = sc_ps.tile([P, CHW], F32, tag="sc", name="ps_warm")
            for _w in range(WARMUP):
                nc.tensor.matmul(
                    ps_warm[:], ones_bf[:], warm_src[:], start=True, stop=True,
                )

            # k chunk tiles [P, CHW] f32r; lhsT slices are [:, 128i:128i+128]
            kt_ch = [None] * NCH
            v_tiles = [None] * (NCH * SPC)
            q_chs = [None] * NCH
            accs = [None] * NCH
            o_banks = [None] * NCH
            d_banks = [None] * NCH

            def emit_x_dma(j):
                tiles = []
                tsl = slice(CHW * j, CHW * (j + 1))
                for c in range(CT):
                    xt = x_pool.tile([P, CHW], F32R, tag=f"xc{c}", name=f"x{j}_{c}")
                    nc.sync.dma_start(xt[:], xT_r[:, c, tsl])
                    tiles.append(xt)
                return tiles

            def proj_units(j, x_tiles):
                """Thunk list: 24 proj matmuls + copy tails + v transposes."""
                units = []
                ps_tiles = {}

                def mk_mm(kind, wname, c):
                    def f():
                        if c == 0:
                            ps_tiles[kind] = proj_ps.tile(
                                [P, CHW], F32, tag="proj", name=f"ps_{kind}{j}"
                            )
                        nc.tensor.matmul(
                            ps_tiles[kind][:],
                            w_sb[wname][c][:],
                            x_tiles[c][:],
                            start=(c == 0),
                            stop=(c == CT - 1),
                        )
                    return f

                def q_tail():
                    q = qv_pool.tile([P, CHW], F32R, tag="qch", name=f"q{j}")
                    nc.scalar.activation(
                        q[:], ps_tiles["q"][:], Identity, bias=b_sb["bq"][:]
                    )
                    q_chs[j] = q

                def k_tail():
                    kt = kt_pool.tile([P, CHW], F32R, tag="ktch", name=f"kt{j}")
                    nc.vector.tensor_copy(kt[:], ps_tiles["k"][:])
                    kt_ch[j] = kt

                def v_tail():
                    vch = qv_pool.tile([P, CHW], BF16, tag="vch", name=f"v{j}")
                    nc.scalar.activation(
                        vch[:], ps_tiles["v"][:], Identity, bias=b_sb["bv"][:]
                    )
                    ps_tiles["vch"] = vch

                def mk_vt(st):
                    def f():
                        ps_t = tr_ps.tile([P, P], BF16, tag="tr")
                        nc.tensor.transpose(
                            ps_t[:],
                            ps_tiles["vch"][:, P * st : P * (st + 1)],
                            ident[:],
                        )
                        vt = v_pool.tile(
                            [P, P], BF16, tag="vnat", name=f"vnat_{SPC*j+st}"
                        )
                        nc.vector.tensor_copy(vt[:], ps_t[:])
                        v_tiles[SPC * j + st] = vt
                    return f

                for kind, wname, tail in (
                    ("q", "Wq", q_tail),
                    ("k", "Wk", k_tail),
                    ("v", "Wv", v_tail),
                ):
                    for c in range(CT):
                        units.append(mk_mm(kind, wname, c))
                    units.append(tail)
                tr_units = [mk_vt(st) for st in range(SPC)]
                return units, tr_units

            def emit_attn(j, fill_units):
                """Attention for chunk j, interleaving fill_units into the PE
                stream to cover exp latency. Denominator: off-diagonal tiles
                accumulate on GpSimd (off the critical path); diagonal tiles
                go straight to ps_d via trimmed ones-matmuls on the PE."""
                n_s = SPC * (j + 1)
                n_off = SPC * j
                q_ch = q_chs[j]
                if n_off:
                    acc = acc_pool.tile([P, CHW], F32R, tag="dacc", name=f"acc{j}")
                    accs[j] = acc
                ps_o = o_ps.tile([P, CHW], F32, tag="o", name=f"o{j}")
                o_banks[j] = ps_o
                ps_d = d_ps.tile([P, CHW], F32, tag="d", name=f"d{j}")
                d_banks[j] = ps_d

                nu = len(fill_units)
                pending = [None]

                def emit_pending():
                    if pending[0] is not None:
                        pending[0]()
                        pending[0] = None

                for i in range(n_s):
                    diag = i >= n_off
                    st = i - n_off
                    w0 = P * st if diag else 0
                    w0sc = min(w0, CHW - 256)

                    ps_sc = sc_ps.tile([P, CHW], F32, tag="sc", name=f"sc{j}_{i}")
                    nc.tensor.matmul(
                        ps_sc[:, w0sc:],
                        kt_ch[i // SPC][:, P * (i % SPC) : P * (i % SPC + 1)],
                        q_ch[:, w0sc:],
                        start=True,
                        stop=True,
                    )
                    eb = e_pool.tile([P, CHW], BF16, tag="e", name=f"e{j}_{i}")
                    nc.scalar.activation(eb[:, w0:], ps_sc[:, w0:], Exp)
                    if diag:
                        nc.vector.tensor_tensor(
                            eb[:, w0 : w0 + P], eb[:, w0 : w0 + P], tril[:], Mult
                        )
                    if not diag:
                        if i == 0:
                            nc.gpsimd.tensor_copy(acc[:], eb[:])
                        else:
                            nc.gpsimd.tensor_tensor(acc[:], acc[:], eb[:], Add)

                    lo = nu * i // n_s
                    hi = nu * (i + 1) // n_s
                    for u in range(lo, min(lo + 2, hi)):
                        fill_units[u]()
                    emit_pending()
                    for u in range(min(lo + 2, hi), hi):
                        fill_units[u]()

                    vt = v_tiles[i]
                    ii = i

                    def pend(eb=eb, vt=vt, w0=w0, ii=ii, diag=diag, st=st):
                        nc.tensor.matmul(
                            ps_o[:, w0:],
                            vt[:],
                            eb[:, w0:],
                            start=(ii == 0),
                            stop=(ii == n_s - 1),
                        )
                        if diag:
                            if st == 0 and n_off:
                                # fold the GpSimd-accumulated off-diag sum in
                                nc.tensor.matmul(
                                    ps_d[:], ones_r[:], acc[:],
                                    start=True, stop=False,
                                )
                            nc.tensor.matmul(
                                ps_d[:, w0:], ones_bf[:], eb[:, w0:],
                                start=(st == 0 and not n_off),
                                stop=(st == SPC - 1),
                            )
                    pending[0] = pend
                emit_pending()

            def emit_end(j, nsplit):
                tsl0 = CHW * j
                ps_d = d_banks[j]
                recip = out_pool.tile([P, CHW], F32, tag="recip", name=f"rc{j}")
                o_sb = out_pool.tile([P, CHW], F32, tag="osb", name=f"ob{j}")
                w = CHW // nsplit
                for s in range(nsplit):
                    sl = slice(w * s, w * (s + 1))
                    nc.vector.reciprocal_approx_fast(
                        out=recip[:, sl], in_=ps_d[:, sl]
                    )
                    nc.vector.tensor_tensor(
                        o_sb[:, sl], o_banks[j][:, sl], recip[:, sl], Mult
                    )
                    nc.sync.dma_start(
                        out_ext[:, tsl0 + w * s : tsl0 + w * (s + 1)], o_sb[:, sl]
                    )

            # ---- main schedule ----
            units, tr_units = proj_units(0, x0_tiles)
            for u in units:
                u()
            for j in range(NCH):
                if j + 1 < NCH:
                    x_next = emit_x_dma(j + 1)
                    fill, tr_next = proj_units(j + 1, x_next)
                else:
                    fill, tr_next = [], []
                emit_attn(j, tr_units + fill)
                tr_units = tr_next
                emit_end(j, 2 if j + 1 < NCH else 4)

    nc.compile()
    _BUILT = nc
    return nc


def _host_inputs(x, Wq, bq, Wk, bk, Wv, bv):
    import ml_dtypes

    tril = (np.arange(P)[:, None] <= np.arange(P)[None, :]).astype(
        ml_dtypes.bfloat16
    )
    shared = {
        "Wq": np.ascontiguousarray(Wq, dtype=np.float32),
        "Wk": np.ascontiguousarray(Wk, dtype=np.float32),
        "Wv": np.ascontiguousarray(Wv, dtype=np.float32),
        "bq": np.ascontiguousarray(bq, dtype=np.float32).reshape(H, 1),
        "bv": np.ascontiguousarray(bv, dtype=np.float32).reshape(H, 1),
        "tril": tril,
        "ones": np.ones((P, P), dtype=np.float32),
        "ident": np.eye(P, dtype=np.float32).astype(ml_dtypes.bfloat16),
    }
    in_maps = []
    for b in range(B):
        m = dict(shared)
        m["xT"] = np.ascontiguousarray(np.asarray(x[b], dtype=np.float32).T)
        in_maps.append(m)
    return in_maps


def kernel(x, Wq, bq, Wk, bk, Wv, bv):
    global LAST_EXEC_TIME_NS
    from concourse.bass_utils import run_bass_kernel_spmd

    nc = _build()
    in_maps = _host_inputs(x, Wq, bq, Wk, bk, Wv, bv)
    trace = os.environ.get("BASS_ATTN_TRACE", "0") == "1"
    res = run_bass_kernel_spmd(nc, in_maps, core_ids=list(range(N_CORES)), trace=trace)
    LAST_EXEC_TIME_NS = res.exec_time_ns
    out = np.stack([res.results[b]["out"].T for b in range(B)], axis=0)
    return np.ascontiguousarray(out, dtype=np.float32)
